# revision 21
# baseline (speedup 1.0000x reference)
"""DNPU layer (128 independent per-expert MLPs, batch 16384) on 8 trn2 cores.

Sharding: expert-parallel — core k owns experts 16k..16k+15 and the full
batch. Host-side prep folds the control electrodes and every bias into the
weight blocks (ones-row trick), transposes x into an electrode-major layout,
and pads H=90 -> 91 so each layer is a pure matmul -> relu chain on device.

Device program (identical on all 8 cores, different data):
  for each 1024-wide batch pair:
    layer-major sweeps over the 16 experts:
      L0: [64 x 1024] x-tile  @ [64 x 91] block-diag W  -> psum -> relu -> h0
      L1..L3: h @ [91 x 91] -> psum -> relu -> h(l+1)   (bias in row 90)
      L4: h3 @ [91 x 1] -> psum row 32*(n%4)            (bias folded, M=1)
    out rows DMA'd straight from PSUM to DRAM.
Matmul operands are bitcast to float32r (full-rate fp32 PE mode).
"""

import sys

if "/opt/trn_rl_repo" not in sys.path:
    sys.path.insert(0, "/opt/trn_rl_repo")

from contextlib import ExitStack

import numpy as np

import concourse.bass as bass
import concourse.mybir as mybir
import concourse.tile as tile
from concourse.bass import ds, ts

B = 16384  # batch
N = 128  # experts
I = 3  # data electrodes / expert
C = 4  # control electrodes / expert
H = 90  # hidden width
L = 3  # extra hidden layers
NCORES = 8
NLOC = N // NCORES  # 16 experts per core
M1 = H + 1  # padded hidden width (ones/bias row)
F = 512  # matmul moving free dim (fp32 max / one psum bank)
PAIR = 2 * F  # pointwise tile width

# All matmuls are padded to K=128 (contraction) and M=128: K<128 runs the
# PE at half stream rate (measured 438 vs 229 ns per N=512 matmul). Zero
# weight rows/cols make the padding numerically free, and M=128 keeps every
# psum/h tile fully written (CoreSim-clean K=128 chaining).
BLK = 128
W0_W = NLOC * BLK  # 2048
WH_W = L * NLOC * BLK  # 6144
WO_W = NLOC * NLOC  # 256 (block-diag out weights)
WALL_W = W0_W + WH_W + WO_W

# Matmul operand dtype: "bf16" streams 1 col/cycle (213 ns per N=512 mm),
# "f32r" streams at half rate (427 ns, measured), "f32" quarter rate.
MM_DTYPE = "f32r"  # 242 ns/mm at K=128 (vs bf16 229), rel err 4e-4 vs 7e-3


def build_nc(b=B, mm_dtype=None, h_bufs=33):
    """Build the single-core Bass program (SPMD across cores via data)."""
    npair = b // PAIR
    f32 = mybir.dt.float32
    mmdt = {
        "bf16": mybir.dt.bfloat16,
        "f32r": mybir.dt.float32r,
        "f32": f32,
    }[mm_dtype or MM_DTYPE]

    nc = bass.Bass("TRN2", target_bir_lowering=False, debug=False)
    xTr = nc.dram_tensor("xTr", [BLK, b], mmdt, kind="ExternalInput").ap()
    wall = nc.dram_tensor("wall", [128, WALL_W], mmdt, kind="ExternalInput").ap()
    outT = nc.dram_tensor("outT", [NLOC, b], f32, kind="ExternalOutput").ap()

    from concourse.tile import add_dep_helper

    with ExitStack() as ctx:
        tc = ctx.enter_context(tile.TileContext(nc))
        wpool = ctx.enter_context(tc.tile_pool(name="w", bufs=1))
        xpool = ctx.enter_context(tc.tile_pool(name="x", bufs=3))
        hpool = ctx.enter_context(tc.tile_pool(name="h", bufs=h_bufs))
        # One shared PSUM pool for layer tiles AND output tiles:
        # 8 bufs x [128, 512] fp32 = exactly all 8 PSUM banks.
        pspool = ctx.enter_context(tc.tile_pool(name="ps", bufs=8, space="PSUM"))
        ospool = ctx.enter_context(tc.tile_pool(name="os", bufs=3))

        wall_sb = wpool.tile([128, WALL_W], mmdt)
        dma_wall = nc.sync.dma_start(wall_sb[:], wall[:])
        # PE NOP probe: absorbs the weight-DMA queue sem into PE's observed
        # clock (matmuls have a 1-sync-wait codegen budget, and Tile sync is
        # not transitive across processors).
        nop_w = nc.tensor.nop()
        add_dep_helper(nop_w.ins, dma_wall.ins, reason="absorb wall dma wait")

        # Engine assignment for the PSUM->SBUF pointwise ops.
        # A hidden matmul at psum-alloc c waits on the pointwise of alloc
        # c-32 (its rhs half producer) and of alloc c-8 (psum slot WAR,
        # bufs=8). Per pair (130 allocs: 128 relus + 2 out-copies) that
        # links alloc a with a+24 for a in 0..95 -> 24 chains over allocs
        # 0..119; each chain stays on one engine so those matmuls need only
        # ONE sem wait. Allocs 120..129 are free. 14 ACT / 10 DVE chains +
        # all-DVE frees balances the engines (ACT 570 ns vs DVE 658 ns/op).
        chain_eng = [0, 0, 0, 1, 0, 1, 0, 1, 0, 1, 0, 1,
                     0, 0, 0, 1, 0, 1, 0, 1, 0, 1, 0, 1]  # 14 ACT, 10 DVE
        alloc_cnt = [0]
        pw_insts = {}

        def psum_alloc():
            c = alloc_cnt[0]
            t = pspool.tile([128, F], mybir.dt.float32, name="ps", tag="ps")
            alloc_cnt[0] += 1
            return t, c

        def engine_for(c):
            a = c % 130
            return chain_eng[a % 24] if a < 120 else 1

        def pointwise(dst, src, relu, c):
            if engine_for(c) == 0:
                func = (
                    mybir.ActivationFunctionType.Relu
                    if relu
                    else mybir.ActivationFunctionType.Identity
                )
                ins = nc.scalar.activation(dst, src, func)
            elif relu:
                ins = nc.vector.tensor_scalar_max(dst, src, 0.0)
            else:
                ins = nc.vector.tensor_copy(dst, src)
            pw_insts[c] = ins

        for p in range(npair):
            xt = xpool.tile([BLK, PAIR], mmdt)
            dma_xt = nc.sync.dma_start(xt[:], xTr[:, ts(p, PAIR)])
            nop_x = nc.tensor.nop()
            add_dep_helper(nop_x.ins, dma_xt.ins, reason="absorb xt dma wait")

            hs = [[None] * (L + 1) for _ in range(NLOC)]
            # layer sweeps (l=0 reads xt, l>=1 read h tiles)
            for l in range(L + 1):
                for n in range(NLOC):
                    if l == 0:
                        lhsT = wall_sb[:, ds(n * BLK, BLK)]
                        rhs = xt
                    else:
                        lhsT = wall_sb[:, ds(W0_W + ((l - 1) * NLOC + n) * BLK, BLK)]
                        rhs = hs[n][l - 1]
                    ht = hpool.tile([BLK, PAIR], mmdt, tag="h")
                    for v in range(2):
                        ps, c = psum_alloc()
                        nc.tensor.matmul(ps[:], lhsT, rhs[:, ts(v, F)])
                        pointwise(ht[:, ts(v, F)], ps[:], True, c)
                    hs[n][l] = ht
            # output sweep: all 16 experts accumulate into two [16, F] psum
            # tiles (one per half); expert n's lhsT [128, 16] has only
            # column n nonzero, so start/stop-chained matmuls add +0
            # elsewhere. Two copies + one DMA per pair.
            c_ot0 = alloc_cnt[0]
            if c_ot0 >= 8:
                nop_o = nc.tensor.nop()
                add_dep_helper(
                    nop_o.ins, pw_insts[c_ot0 - 8].ins,
                    reason="absorb ot slot WAR",
                )
            ots = [psum_alloc() for _ in range(2)]
            for n in range(NLOC):
                lhsT = wall_sb[:, ds(W0_W + WH_W + NLOC * n, NLOC)]
                h3 = hs[n][L]
                for v in range(2):
                    nc.tensor.matmul(
                        ots[v][0][0:NLOC, :],
                        lhsT,
                        h3[:, ts(v, F)],
                        start=(n == 0),
                        stop=(n == NLOC - 1),
                    )
            os_t = ospool.tile([NLOC, PAIR], f32, name="os", tag="os")
            for v in range(2):
                pointwise(os_t[:, ts(v, F)], ots[v][0][0:NLOC, :], False, ots[v][1])
            nc.sync.dma_start(outT[:, ts(p, PAIR)], os_t[:])
    return nc


def _split_excess_waits(bir_bytes: bytes) -> bytes:
    """BIR post-pass: walrus codegen allows at most ONE sync wait per engine
    instruction; hoist extra waits onto inserted no-update NoOps."""
    import json as _json

    d = _json.loads(bir_bytes)
    ctr = 0
    for fn in d.get("functions", []):
        for bb in fn.get("blocks", []):
            out = []
            for ins in bb.get("instructions", []):
                si = ins.get("sync_info") or {}
                ow = si.get("on_wait") or []
                if len(ow) > 1 and ins.get("engine"):
                    for w in ow[:-1]:
                        ctr += 1
                        out.append(
                            {
                                "debug": ins.get("debug", 0),
                                "engine": ins["engine"],
                                "ins": [],
                                "outs": [],
                                "name": f"WSPLIT-{ctr}",
                                "opcode": "NoOp",
                                "sync_info": {"on_update": [], "on_wait": [w]},
                            }
                        )
                    si["on_wait"] = [ow[-1]]
                out.append(ins)
            bb["instructions"] = out
    return _json.dumps(d).encode()


def install_wait_splitter():
    from concourse import bass2jax, bass_utils

    if getattr(bass_utils, "_ws_installed", False):
        return
    orig = bass_utils.compile_bir_kernel

    def patched(bir_json, tmpdir, neff_name="file.neff"):
        return orig(_split_excess_waits(bir_json), tmpdir, neff_name=neff_name)

    bass_utils.compile_bir_kernel = patched
    bass2jax.compile_bir_kernel = patched
    bass_utils._ws_installed = True


def prep_core_inputs(x, controls, W_in, b_in, W_hid, b_hid, W_out, b_out, b=B, mm_dtype=None):
    """Host-side fold + shard: list of per-core input dicts."""
    x = np.asarray(x, np.float32)
    controls = np.asarray(controls, np.float64)
    W_in = np.asarray(W_in, np.float64)
    b_in = np.asarray(b_in, np.float64)
    W_hid = np.asarray(W_hid, np.float32)
    b_hid = np.asarray(b_hid, np.float32)
    W_out = np.asarray(W_out, np.float32)
    b_out = np.asarray(b_out, np.float32)

    # controls fold: beff0[n] = controls[n] @ W_in[n, I:, :] + b_in[n]
    beff0 = (
        np.einsum("nc,nch->nh", controls, W_in[:, I:, :]) + b_in
    ).astype(np.float32)
    W_in_d = W_in[:, :I, :].astype(np.float32)  # [N, 3, H]

    # x transposed to electrode-major with ones rows, zero-padded to 128
    xT = np.ascontiguousarray(x.T).reshape(N, I, b)  # [N, 3, B]
    xTr = np.zeros((NCORES, BLK, b), np.float32)
    v = xTr[:, : 4 * NLOC, :].reshape(NCORES, NLOC, 4, b)
    v[:, :, :I, :] = xT.reshape(NCORES, NLOC, I, b)
    v[:, :, I, :] = 1.0

    in_maps = []
    for k in range(NCORES):
        g0 = k * NLOC
        wallm = np.zeros((128, WALL_W), np.float32)
        w0blk = wallm[:, :W0_W]
        whblk = wallm[:, W0_W : W0_W + WH_W]
        woblk = wallm[:, W0_W + WH_W :]
        for n in range(NLOC):
            g = g0 + n
            w0blk[4 * n : 4 * n + I, n * BLK : n * BLK + H] = W_in_d[g]
            w0blk[4 * n + I, n * BLK : n * BLK + H] = beff0[g]
            w0blk[4 * n + I, n * BLK + H] = 1.0
        for l in range(L):
            for n in range(NLOC):
                g = g0 + n
                base = (l * NLOC + n) * BLK
                whblk[:H, base : base + H] = W_hid[l, g]
                whblk[H, base : base + H] = b_hid[l, g]
                whblk[H, base + H] = 1.0
        woblk[:H, 0 :: NLOC + 1] = W_out[g0 : g0 + NLOC].T
        woblk[H, 0 :: NLOC + 1] = b_out[g0 : g0 + NLOC]
        npdt = mybir.dt.np(
            {"bf16": mybir.dt.bfloat16, "f32r": mybir.dt.float32r, "f32": mybir.dt.float32}[
                mm_dtype or MM_DTYPE
            ]
        )
        in_maps.append(
            {
                "xTr": np.ascontiguousarray(xTr[k]).astype(npdt),
                "wall": wallm.astype(npdt),
            }
        )
    return in_maps


def run_sharded(inputs, b=B, mm_dtype=None, trace=False, **kw):
    """Build + run on the 8 cores; returns (out [b, N] fp32, BassKernelResults)."""
    from concourse import bass_utils

    install_wait_splitter()
    nc = build_nc(b=b, mm_dtype=mm_dtype)
    in_maps = prep_core_inputs(b=b, mm_dtype=mm_dtype, **inputs)
    res = bass_utils.run_bass_kernel_spmd(
        nc, in_maps, core_ids=list(range(NCORES)), trace=trace, **kw
    )
    out = np.empty((b, N), np.float32)
    for k in range(NCORES):
        out[:, k * NLOC : (k + 1) * NLOC] = res.results[k]["outT"].T
    return out, res


def kernel(**inputs) -> np.ndarray:
    out, _ = run_sharded(inputs)
    return out


# revision 22
# speedup vs baseline: 1.0412x; 1.0412x over previous
"""DNPU layer (128 independent per-expert MLPs, batch 16384) on 8 trn2 cores.

Sharding: expert-parallel — core k owns experts 16k..16k+15 and the full
batch. Host-side prep folds the control electrodes and every bias into the
weight blocks (ones-row trick), transposes x into an electrode-major layout,
and pads H=90 -> 91 so each layer is a pure matmul -> relu chain on device.

Device program (identical on all 8 cores, different data):
  for each 1024-wide batch pair:
    layer-major sweeps over the 16 experts:
      L0: [64 x 1024] x-tile  @ [64 x 91] block-diag W  -> psum -> relu -> h0
      L1..L3: h @ [91 x 91] -> psum -> relu -> h(l+1)   (bias in row 90)
      L4: h3 @ [91 x 1] -> psum row 32*(n%4)            (bias folded, M=1)
    out rows DMA'd straight from PSUM to DRAM.
Matmul operands are bitcast to float32r (full-rate fp32 PE mode).
"""

import sys

if "/opt/trn_rl_repo" not in sys.path:
    sys.path.insert(0, "/opt/trn_rl_repo")

from contextlib import ExitStack

import numpy as np

import concourse.bass as bass
import concourse.mybir as mybir
import concourse.tile as tile
from concourse.bass import ds, ts

B = 16384  # batch
N = 128  # experts
I = 3  # data electrodes / expert
C = 4  # control electrodes / expert
H = 90  # hidden width
L = 3  # extra hidden layers
NCORES = 8
NLOC = N // NCORES  # 16 experts per core
M1 = H + 1  # padded hidden width (ones/bias row)
F = 512  # matmul moving free dim (fp32 max / one psum bank)
PAIR = 2 * F  # pointwise tile width

# All matmuls are padded to K=128 (contraction) and M=128: K<128 runs the
# PE at half stream rate (measured 438 vs 229 ns per N=512 matmul). Zero
# weight rows/cols make the padding numerically free, and M=128 keeps every
# psum/h tile fully written (CoreSim-clean K=128 chaining).
BLK = 128
W0_W = NLOC * BLK  # 2048
WH_W = L * NLOC * BLK  # 6144
WO_W = NLOC * NLOC  # 256 (block-diag out weights)
WALL_W = W0_W + WH_W + WO_W

# Matmul operand dtype: "bf16" streams 1 col/cycle (213 ns per N=512 mm),
# "f32r" streams at half rate (427 ns, measured), "f32" quarter rate.
MM_DTYPE = "f32r"  # 242 ns/mm at K=128 (vs bf16 229), rel err 4e-4 vs 7e-3


def build_nc(b=B, mm_dtype=None, h_bufs=33):
    """Build the single-core Bass program (SPMD across cores via data)."""
    npair = b // PAIR
    f32 = mybir.dt.float32
    mmdt = {
        "bf16": mybir.dt.bfloat16,
        "f32r": mybir.dt.float32r,
        "f32": f32,
    }[mm_dtype or MM_DTYPE]

    nc = bass.Bass("TRN2", target_bir_lowering=False, debug=False)
    xTr = nc.dram_tensor("xTr", [BLK, b], mmdt, kind="ExternalInput").ap()
    wall = nc.dram_tensor("wall", [128, WALL_W], mmdt, kind="ExternalInput").ap()
    outT = nc.dram_tensor("outT", [NLOC, b], f32, kind="ExternalOutput").ap()

    from concourse.tile import add_dep_helper

    with ExitStack() as ctx:
        tc = ctx.enter_context(tile.TileContext(nc))
        wpool = ctx.enter_context(tc.tile_pool(name="w", bufs=1))
        xpool = ctx.enter_context(tc.tile_pool(name="x", bufs=3))
        hpool = ctx.enter_context(tc.tile_pool(name="h", bufs=h_bufs))
        # One shared PSUM pool for layer tiles AND output tiles:
        # 4 bufs x [128, 1024] fp32 = exactly all 8 PSUM banks.
        pspool = ctx.enter_context(tc.tile_pool(name="ps", bufs=4, space="PSUM"))
        ospool = ctx.enter_context(tc.tile_pool(name="os", bufs=3))

        wall_sb = wpool.tile([128, WALL_W], mmdt)
        dma_wall = nc.sync.dma_start(wall_sb[:], wall[:])
        # PE NOP probe: absorbs the weight-DMA queue sem into PE's observed
        # clock (matmuls have a 1-sync-wait codegen budget, and Tile sync is
        # not transitive across processors).
        nop_w = nc.tensor.nop()
        add_dep_helper(nop_w.ins, dma_wall.ins, reason="absorb wall dma wait")

        # Engine assignment for the PSUM->SBUF pointwise ops.
        # A hidden matmul at psum-alloc c waits on the pointwise of alloc
        # c-16 (its rhs producer) and of alloc c-4 (psum slot WAR, bufs=4).
        # Per pair (65 allocs: 64 relus + 1 out-copy) that links alloc a
        # with a+12 for a in 0..47 -> 12 chains over allocs 0..59; each
        # chain stays on one engine so those matmuls need only ONE sem wait
        # (matmul codegen budget). Allocs 60..64 are free. Alternating
        # chains avoid engine bursts; measured op costs (FD=1024): ACT
        # ~1107 ns, DVE ~1219 ns -> 6+6 chains with 4 ACT / 1 DVE frees.
        chain_eng = [0, 1, 0, 1, 0, 1, 0, 1, 0, 1, 0, 1]
        free_eng = [0, 0, 0, 0, 1]  # allocs 60..64
        alloc_cnt = [0]
        pw_insts = {}

        def psum_alloc():
            c = alloc_cnt[0]
            t = pspool.tile([128, PAIR], mybir.dt.float32, name="ps", tag="ps")
            alloc_cnt[0] += 1
            return t, c

        def engine_for(c):
            a = c % 65
            return chain_eng[a % 12] if a < 60 else free_eng[a - 60]

        def pointwise(dst, src, relu, c):
            if engine_for(c) == 0:
                func = (
                    mybir.ActivationFunctionType.Relu
                    if relu
                    else mybir.ActivationFunctionType.Identity
                )
                ins = nc.scalar.activation(dst, src, func)
            elif relu:
                ins = nc.vector.tensor_scalar_max(dst, src, 0.0)
            else:
                ins = nc.vector.tensor_copy(dst, src)
            pw_insts[c] = ins

        for p in range(npair):
            xt = xpool.tile([BLK, PAIR], mmdt)
            dma_xt = nc.sync.dma_start(xt[:], xTr[:, ts(p, PAIR)])
            nop_x = nc.tensor.nop()
            add_dep_helper(nop_x.ins, dma_xt.ins, reason="absorb xt dma wait")

            hs = [[None] * (L + 1) for _ in range(NLOC)]
            # layer sweeps (l=0 reads xt, l>=1 read h tiles)
            for l in range(L + 1):
                for n in range(NLOC):
                    if l == 0:
                        lhsT = wall_sb[:, ds(n * BLK, BLK)]
                        rhs = xt
                    else:
                        lhsT = wall_sb[:, ds(W0_W + ((l - 1) * NLOC + n) * BLK, BLK)]
                        rhs = hs[n][l - 1]
                    ps, c = psum_alloc()
                    for v in range(2):
                        nc.tensor.matmul(ps[:, ts(v, F)], lhsT, rhs[:, ts(v, F)])
                    ht = hpool.tile([BLK, PAIR], mmdt, tag="h")
                    pointwise(ht[:], ps[:], True, c)
                    hs[n][l] = ht
            # output sweep: all 16 experts accumulate into ONE [16, PAIR]
            # psum tile; expert n's lhsT [128, 16] has only column n nonzero,
            # so start/stop-chained matmuls add +0 elsewhere. One copy + one
            # DMA per pair.
            c_ot = alloc_cnt[0]
            if c_ot >= 4:
                nop_o = nc.tensor.nop()
                add_dep_helper(
                    nop_o.ins, pw_insts[c_ot - 4].ins,
                    reason="absorb ot slot WAR",
                )
            ot, cot = psum_alloc()
            for n in range(NLOC):
                lhsT = wall_sb[:, ds(W0_W + WH_W + NLOC * n, NLOC)]
                h3 = hs[n][L]
                for v in range(2):
                    nc.tensor.matmul(
                        ot[0:NLOC, ts(v, F)],
                        lhsT,
                        h3[:, ts(v, F)],
                        start=(n == 0),
                        stop=(n == NLOC - 1),
                    )
            os_t = ospool.tile([NLOC, PAIR], f32, name="os", tag="os")
            pointwise(os_t[:], ot[0:NLOC, :], False, cot)
            nc.sync.dma_start(outT[:, ts(p, PAIR)], os_t[:])
    return nc


def _split_excess_waits(bir_bytes: bytes) -> bytes:
    """BIR post-pass: walrus codegen allows at most ONE sync wait per engine
    instruction; hoist extra waits onto inserted no-update NoOps."""
    import json as _json

    d = _json.loads(bir_bytes)
    ctr = 0
    for fn in d.get("functions", []):
        for bb in fn.get("blocks", []):
            out = []
            for ins in bb.get("instructions", []):
                si = ins.get("sync_info") or {}
                ow = si.get("on_wait") or []
                if len(ow) > 1 and ins.get("engine"):
                    for w in ow[:-1]:
                        ctr += 1
                        out.append(
                            {
                                "debug": ins.get("debug", 0),
                                "engine": ins["engine"],
                                "ins": [],
                                "outs": [],
                                "name": f"WSPLIT-{ctr}",
                                "opcode": "NoOp",
                                "sync_info": {"on_update": [], "on_wait": [w]},
                            }
                        )
                    si["on_wait"] = [ow[-1]]
                out.append(ins)
            bb["instructions"] = out
    return _json.dumps(d).encode()


def install_wait_splitter():
    from concourse import bass2jax, bass_utils

    if getattr(bass_utils, "_ws_installed", False):
        return
    orig = bass_utils.compile_bir_kernel

    def patched(bir_json, tmpdir, neff_name="file.neff"):
        return orig(_split_excess_waits(bir_json), tmpdir, neff_name=neff_name)

    bass_utils.compile_bir_kernel = patched
    bass2jax.compile_bir_kernel = patched
    bass_utils._ws_installed = True


def prep_core_inputs(x, controls, W_in, b_in, W_hid, b_hid, W_out, b_out, b=B, mm_dtype=None):
    """Host-side fold + shard: list of per-core input dicts."""
    x = np.asarray(x, np.float32)
    controls = np.asarray(controls, np.float64)
    W_in = np.asarray(W_in, np.float64)
    b_in = np.asarray(b_in, np.float64)
    W_hid = np.asarray(W_hid, np.float32)
    b_hid = np.asarray(b_hid, np.float32)
    W_out = np.asarray(W_out, np.float32)
    b_out = np.asarray(b_out, np.float32)

    # controls fold: beff0[n] = controls[n] @ W_in[n, I:, :] + b_in[n]
    beff0 = (
        np.einsum("nc,nch->nh", controls, W_in[:, I:, :]) + b_in
    ).astype(np.float32)
    W_in_d = W_in[:, :I, :].astype(np.float32)  # [N, 3, H]

    # x transposed to electrode-major with ones rows, zero-padded to 128
    xT = np.ascontiguousarray(x.T).reshape(N, I, b)  # [N, 3, B]
    xTr = np.zeros((NCORES, BLK, b), np.float32)
    v = xTr[:, : 4 * NLOC, :].reshape(NCORES, NLOC, 4, b)
    v[:, :, :I, :] = xT.reshape(NCORES, NLOC, I, b)
    v[:, :, I, :] = 1.0

    in_maps = []
    for k in range(NCORES):
        g0 = k * NLOC
        wallm = np.zeros((128, WALL_W), np.float32)
        w0blk = wallm[:, :W0_W]
        whblk = wallm[:, W0_W : W0_W + WH_W]
        woblk = wallm[:, W0_W + WH_W :]
        for n in range(NLOC):
            g = g0 + n
            w0blk[4 * n : 4 * n + I, n * BLK : n * BLK + H] = W_in_d[g]
            w0blk[4 * n + I, n * BLK : n * BLK + H] = beff0[g]
            w0blk[4 * n + I, n * BLK + H] = 1.0
        for l in range(L):
            for n in range(NLOC):
                g = g0 + n
                base = (l * NLOC + n) * BLK
                whblk[:H, base : base + H] = W_hid[l, g]
                whblk[H, base : base + H] = b_hid[l, g]
                whblk[H, base + H] = 1.0
        woblk[:H, 0 :: NLOC + 1] = W_out[g0 : g0 + NLOC].T
        woblk[H, 0 :: NLOC + 1] = b_out[g0 : g0 + NLOC]
        npdt = mybir.dt.np(
            {"bf16": mybir.dt.bfloat16, "f32r": mybir.dt.float32r, "f32": mybir.dt.float32}[
                mm_dtype or MM_DTYPE
            ]
        )
        in_maps.append(
            {
                "xTr": np.ascontiguousarray(xTr[k]).astype(npdt),
                "wall": wallm.astype(npdt),
            }
        )
    return in_maps


def run_sharded(inputs, b=B, mm_dtype=None, trace=False, **kw):
    """Build + run on the 8 cores; returns (out [b, N] fp32, BassKernelResults)."""
    from concourse import bass_utils

    install_wait_splitter()
    nc = build_nc(b=b, mm_dtype=mm_dtype)
    in_maps = prep_core_inputs(b=b, mm_dtype=mm_dtype, **inputs)
    res = bass_utils.run_bass_kernel_spmd(
        nc, in_maps, core_ids=list(range(NCORES)), trace=trace, **kw
    )
    out = np.empty((b, N), np.float32)
    for k in range(NCORES):
        out[:, k * NLOC : (k + 1) * NLOC] = res.results[k]["outT"].T
    return out, res


def kernel(**inputs) -> np.ndarray:
    out, _ = run_sharded(inputs)
    return out
